# revision 1
# baseline (speedup 1.0000x reference)
"""Masked dot-product attention (B=64, L=1024, D=64, fp32) on 8 NeuronCores.

Strategy (data-parallel over batch, per the sharding hint):
  - Batches are sorted by valid_len (descending) and dealt round-robin to the
    8 cores so every core gets one batch from each of 8 "rank groups"; the
    per-slot key-block loop count is baked at build time as the max over that
    slot's rank group.  Key blocks that are entirely masked are never computed.
  - Scores are computed transposed, S^T[k, q] = K @ Q^T, via
    matmul(lhsT=K^T_slice, rhs=Q^T) so that the softmax axis (k) lands on the
    partition dim.  Q and K are passed pre-transposed [D, L] per batch (host
    layout choice at shard time; there is no 4-byte DMA transpose on TRN2).
  - The sequence mask is fused into the exp: ScalarE computes
    P^T = exp(S^T/8 + bias_k) with a per-partition bias column that is 0 for
    valid keys and -1e6 for masked keys (exp underflows to exactly 0).
  - AV uses V augmented with a ones column: O'^T = [V | 1]^T @ P^T, so row 64
    of the accumulator is the softmax denominator for free.
  - Normalization: VectorE reciprocal of the denominator row, replicated
    across partitions by a step-0 free-dim SBUF->SBUF DMA (PE ones-matmul
    broadcast for the final slot, where latency matters), then one
    VectorE multiply.
All matmuls run in float32r (~1.2e-4 relative error, full PE rate).

Scheduling notes (the in-order engine streams make emission order matter):
  - kb loop is software-pipelined: QK(kb+1) is emitted before AV(kb) so PE
    never parks behind an AV that waits on ScalarE's exp.
  - Pair/slot input DMAs are prefetched one slot ahead; the first pair's
    loads are split so the first QK only waits on ~300KB.
  - The divide epilogue is deferred into the next slot's loop and split into
    independent q-halves to shorten the end-of-kernel serial chain.
"""

import math
from contextlib import ExitStack

import numpy as np

import concourse.bass as bass
import concourse.bacc as bacc
import concourse.mybir as mybir
import concourse.tile as tile
from concourse.bass_utils import run_bass_kernel_spmd

F32 = mybir.dt.float32
F32R = mybir.dt.float32r
EXP = mybir.ActivationFunctionType.Exp

B, L, D = 64, 1024, 64
N_CORES = 8
SLOTS = B // N_CORES  # batches per core
KB = 128              # key-block size (partition dim of S^T)
N_KB = L // KB        # max key blocks
QH = 512              # q chunk per matmul (fp32 moving-operand max)
NQH = L // QH
NEG = -1000000.0


def build_kernel(counts):
    """counts[s] = number of 128-wide key blocks to process for slot s."""
    nc = bacc.Bacc()

    qt_d = nc.dram_tensor("qt", [SLOTS, D, L], F32R, kind="ExternalInput")
    kt_d = nc.dram_tensor("kt", [SLOTS, D, L], F32R, kind="ExternalInput")
    v_d = nc.dram_tensor("v", [SLOTS, L, D + 1], F32R, kind="ExternalInput")
    bias_d = nc.dram_tensor("bias", [KB, SLOTS * N_KB], F32, kind="ExternalInput")
    out_d = nc.dram_tensor("out", [SLOTS, D, L], F32, kind="ExternalOutput")

    with tile.TileContext(nc) as tc, ExitStack() as ctx:
        const_pool = ctx.enter_context(tc.tile_pool(name="const", bufs=1))
        qk_pool = ctx.enter_context(tc.tile_pool(name="qk", bufs=3))
        v_pool = ctx.enter_context(tc.tile_pool(name="v", bufs=4))
        p_pool = ctx.enter_context(tc.tile_pool(name="p", bufs=6))
        ep_pool = ctx.enter_context(tc.tile_pool(name="ep", bufs=4))
        out_pool = ctx.enter_context(tc.tile_pool(name="out", bufs=4))
        psum_s = ctx.enter_context(tc.tile_pool(name="psum_s", bufs=2, space="PSUM"))
        psum_o = ctx.enter_context(tc.tile_pool(name="psum_o", bufs=2, space="PSUM"))

        bias_t = const_pool.tile([KB, SLOTS * N_KB], F32)
        warm_t = const_pool.tile([1, 1], F32)
        ones_t = const_pool.tile([1, D], F32R)

        pair_tiles: dict[int, tuple] = {}
        v_tiles: dict[int, object] = {}
        pair_order = [1, 2, 3, 0]  # big pair last: tail epilogues hide in its long loops
        slot_order = [2 * p + h for p in pair_order for h in range(2)]
        next_pair = {pair_order[i]: pair_order[i + 1] for i in range(len(pair_order) - 1)}
        next_slot = {slot_order[i]: slot_order[i + 1] for i in range(len(slot_order) - 1)}

        def load_pair(p):
            if p in pair_tiles:
                return
            n_max = counts[2 * p]
            # Two batches packed on the partition dim: even batch in
            # partitions 0-63, odd batch in 64-127.
            qt_t = qk_pool.tile([2 * D, L], F32R, tag="qt", name="qt_t")
            kt_t = qk_pool.tile([2 * D, L], F32R, tag="kt", name="kt_t")
            src_q = qt_d[2 * p : 2 * p + 2].rearrange("b d l -> (b d) l")
            src_k = kt_d[2 * p : 2 * p + 2].rearrange("b d l -> (b d) l")
            if not pair_tiles:
                # Piecewise: the first slot's kb-0 QKs only wait on the kt
                # head block + their own 64 qt rows (~320KB, 2 gens).
                nc.sync.dma_start(kt_t[:, :KB], src_k[:, :KB])
                nc.sync.dma_start(qt_t[:D, :], src_q[:D, :])
                nc.sync.dma_start(qt_t[D:, :], src_q[D:, :])
                if n_max > 1:
                    nc.sync.dma_start(
                        kt_t[:, KB : n_max * KB], src_k[:, KB : n_max * KB]
                    )
            else:
                nc.sync.dma_start(qt_t[:], src_q)
                nc.sync.dma_start(kt_t[:, : n_max * KB], src_k[:, : n_max * KB])
            pair_tiles[p] = (qt_t, kt_t)

        def load_v(s):
            if s in v_tiles:
                return
            n_kb = counts[s]
            v_t = v_pool.tile([KB, N_KB, D + 1], F32R, name="v_t")
            nc.gpsimd.dma_start(
                v_t[:, :n_kb, :],
                v_d[s].rearrange("(n p) d -> p n d", p=KB)[:, :n_kb, :],
            )
            v_tiles[s] = v_t

        def qk(s_ps, rows, kt_t, qt_t, kb):
            for qh in range(NQH):
                nc.tensor.matmul(
                    s_ps[:, qh * QH : (qh + 1) * QH],
                    kt_t[rows, kb * KB : (kb + 1) * KB],
                    qt_t[rows, qh * QH : (qh + 1) * QH],
                    start=True,
                    stop=True,
                )

        def make_tail(s, o_ps, rec_b, qh):
            # Deferred epilogue part B for one q-half: divide and store.
            c0, c1 = qh * QH, (qh + 1) * QH

            last = s == slot_order[-1]

            def tail():
                out_sb = out_pool.tile([D, QH], F32, name="out_sb")
                nch = 1
                cw = QH // nch
                for ch in range(nch):
                    nc.vector.tensor_tensor(
                        out_sb[:, ch * cw : (ch + 1) * cw],
                        o_ps[:D, c0 + ch * cw : c0 + (ch + 1) * cw],
                        rec_b[:, c0 + ch * cw : c0 + (ch + 1) * cw],
                        op=mybir.AluOpType.mult,
                    )
                    nc.sync.dma_start(
                        out_d[s][:, c0 + ch * cw : c0 + (ch + 1) * cw],
                        out_sb[:, ch * cw : (ch + 1) * cw],
                    )

            return tail

        load_pair(pair_order[0])
        # bias rides the SWDGE path so the first exp isn't queued behind
        # the HWDGE input loads.
        nc.gpsimd.dma_start(bias_t[:], bias_d[:])
        # Warm the exp table set while the first DMAs run; also build a
        # ones row (exp of 0 * bias) for the tail's PE broadcast.
        nc.scalar.activation(warm_t[:], bias_t[0:1, 0:1], EXP)
        nc.scalar.activation(ones_t[:], bias_t[0:1, :D], EXP, scale=0.0)
        load_v(slot_order[0])

        # Flat (slot, kb) work list, software-pipelined at depth 2 across
        # slot boundaries: the PE stream is QK(i+1), AV(i-1), so PE never
        # refills the pipeline at a slot change and AV only ever consumes
        # an exp that finished a full iteration ago.
        work = [(s, kb) for s in slot_order for kb in range(counts[s])]
        n_work = len(work)
        slot_first = {s: i for i, (s, kb) in reversed(list(enumerate(work)))}
        o_tiles: dict[int, object] = {}
        s_tiles: dict[tuple, object] = {}
        p_tiles: dict[tuple, object] = {}
        pending_tails: list = []
        tail_due: int = -1

        def emit_qk(i):
            s, kb = work[i]
            pair, half = divmod(s, 2)
            if kb == 0:
                # Slot prologue: prefetch upcoming inputs.
                nxt = slot_order.index(s) + 1
                if nxt < SLOTS:
                    load_v(slot_order[nxt])
                    if nxt + 1 < SLOTS:
                        load_v(slot_order[nxt + 1])
                if half == 0 and pair in next_pair:
                    load_pair(next_pair[pair])
                if half == 1 and pair in next_pair and next_pair[pair] in next_pair:
                    load_pair(next_pair[next_pair[pair]])
            qt_t, kt_t = pair_tiles[pair]
            rows = slice(D * half, D * half + D)
            s_tiles[(s, kb)] = psum_s.tile([KB, L], F32, tag="s", name="s_ps")
            qk(s_tiles[(s, kb)], rows, kt_t, qt_t, kb)

        def emit_av(i):
            s, kb = work[i]
            n_kb = counts[s]
            if kb == 0:
                o_tiles[s] = psum_o.tile([D + 1, L], F32, tag="o", name="o_ps")
            o_ps = o_tiles[s]
            p_t = p_tiles.pop((s, kb))
            for qh in range(NQH):
                nc.tensor.matmul(
                    o_ps[:, qh * QH : (qh + 1) * QH],
                    v_tiles[s][:, kb, :],
                    p_t[:, qh * QH : (qh + 1) * QH],
                    start=(kb == 0),
                    stop=(kb == n_kb - 1),
                )
            if kb == n_kb - 1:
                emit_epilogue_a(s)

        def emit_epilogue_a(s):
            # Reciprocal of the denominator row, then partition-replicate.
            nonlocal pending_tails, tail_due
            if pending_tails:
                for t in pending_tails:
                    t()
                pending_tails = []
            o_ps = o_tiles[s]
            last = s == slot_order[-1]
            rdt = F32R if last else F32
            rec_row = ep_pool.tile([1, L], rdt, tag="l", name="rec_row")
            rec_b = ep_pool.tile([D, L], rdt, tag="rec", name="rec_b")
            if not last:
                nc.vector.reciprocal(rec_row[:], o_ps[D : D + 1, :])
                row_ap = rec_row[:]
                bcast_src = bass.AP(
                    row_ap.tensor, row_ap.offset,
                    [list(row_ap.ap)[0], [0, D]] + list(row_ap.ap)[1:],
                )
                nc.gpsimd.dma_start(rec_b[:], bcast_src)
            else:
              for qh in range(NQH):
                c0, c1 = qh * QH, (qh + 1) * QH
                with nc.allow_low_precision("f32r label for PE-broadcast tail"):
                    nc.vector.reciprocal(rec_row[:, c0:c1], o_ps[D : D + 1, c0:c1])
                if last:
                    # Tail: PE broadcast + ScalarE copy (both idle by now;
                    # keeps the serial DVE chain to recip + multiply).
                    bc_ps = psum_s.tile([D, QH], F32, tag="s", name="bc_ps")
                    nc.tensor.matmul(
                        bc_ps[:], ones_t[:], rec_row[:, c0:c1],
                        start=True, stop=True,
                    )
                    nc.scalar.copy(rec_b[:, c0:c1], bc_ps[:])
                else:
                    row_ap = rec_row[:, c0:c1]
                    bcast_src = bass.AP(
                        row_ap.tensor, row_ap.offset,
                        [list(row_ap.ap)[0], [0, D]] + list(row_ap.ap)[1:],
                    )
                    nc.gpsimd.dma_start(rec_b[:, c0:c1], bcast_src)
            pending_tails = [make_tail(s, o_ps, rec_b, qh) for qh in range(NQH)]
            tail_due = min(slot_first.get(slot_order[slot_order.index(s) + 1], 0) + 3
                           if slot_order.index(s) + 1 < SLOTS else 0, n_work - 1)

        emit_qk(0)
        for i in range(n_work):
            if i + 1 < n_work:
                emit_qk(i + 1)
            if pending_tails and i >= tail_due:
                for t in pending_tails:
                    t()
                pending_tails = []
            s, kb = work[i]
            p_tiles[(s, kb)] = p_pool.tile([KB, L], F32R, name="p_t")
            nc.scalar.activation(
                p_tiles[(s, kb)][:],
                s_tiles.pop((s, kb))[:],
                EXP,
                bias=bias_t[:, s * N_KB + kb : s * N_KB + kb + 1],
                scale=1.0 / math.sqrt(D),
            )
            if i >= 1:
                emit_av(i - 1)
        emit_av(n_work - 1)
        for t in pending_tails:
            t()

    nc.finalize()
    return nc


_NC_CACHE: dict[tuple, object] = {}


def _prepare(queries, keys, values, valid_lens):
    queries = np.ascontiguousarray(queries, dtype=np.float32)
    keys = np.ascontiguousarray(keys, dtype=np.float32)
    values = np.ascontiguousarray(values, dtype=np.float32)
    valid_lens = np.asarray(valid_lens)
    assert queries.shape == (B, L, D), queries.shape
    vl = valid_lens.astype(np.int64)

    # Sort batches by valid_len descending; slot s on core c gets the batch
    # of rank s*8 + c.  Each slot's loop count covers the max valid_len in
    # its rank group, so one instruction stream fits all cores.
    order = np.argsort(-vl, kind="stable")
    counts = tuple(
        max(1, math.ceil(int(vl[order[s * N_CORES]]) / KB)) for s in range(SLOTS)
    )
    # Pairs share a K^T tile sized by the even slot; counts are descending.
    nc = _NC_CACHE.get(counts)
    if nc is None:
        nc = build_kernel(counts)
        _NC_CACHE[counts] = nc

    col = np.arange(L)
    in_maps = []
    for c in range(N_CORES):
        batch_idx = [int(order[s * N_CORES + c]) for s in range(SLOTS)]
        qt = np.ascontiguousarray(
            queries[batch_idx].transpose(0, 2, 1)
        )  # [SLOTS, D, L]
        kt = np.ascontiguousarray(keys[batch_idx].transpose(0, 2, 1))
        v = np.concatenate(
            [values[batch_idx], np.ones((SLOTS, L, 1), np.float32)], axis=2
        )
        bias = np.zeros((KB, SLOTS * N_KB), dtype=np.float32)
        for s in range(SLOTS):
            mask = (col >= vl[batch_idx[s]]).astype(np.float32) * NEG  # [L]
            bias[:, s * N_KB : (s + 1) * N_KB] = mask.reshape(N_KB, KB).T
        in_maps.append({"qt": qt, "kt": kt, "v": v, "bias": bias})
    return nc, in_maps, order


def _unshard(res, order):
    out = np.empty((B, L, D), dtype=np.float32)
    for c in range(N_CORES):
        o = res.results[c]["out"]  # [SLOTS, D, L]
        for s in range(SLOTS):
            out[int(order[s * N_CORES + c])] = o[s].T
    return out


def kernel(queries, keys, values, valid_lens):
    nc, in_maps, order = _prepare(queries, keys, values, valid_lens)
    res = run_bass_kernel_spmd(nc, in_maps, core_ids=list(range(N_CORES)))
    return _unshard(res, order)


def trace_run(queries, keys, values, valid_lens):
    """Like kernel() but traced; returns BassKernelResults (for test.py)."""
    nc, in_maps, order = _prepare(queries, keys, values, valid_lens)
    res = run_bass_kernel_spmd(
        nc, in_maps, core_ids=list(range(N_CORES)), trace=True
    )
    res.full_output = _unshard(res, order)
    return res



# revision 5
# speedup vs baseline: 1.1047x; 1.1047x over previous
"""Masked dot-product attention (B=64, L=1024, D=64, fp32) on 8 NeuronCores.

Strategy (data-parallel over batch, per the sharding hint):
  - Batches are sorted by valid_len (descending) and dealt round-robin to the
    8 cores so every core gets one batch from each of 8 "rank groups"; the
    per-slot key-block loop count is baked at build time as the max over that
    slot's rank group.  Key blocks that are entirely masked are never computed.
  - Scores are computed transposed, S^T[k, q] = K @ Q^T, via
    matmul(lhsT=K^T_slice, rhs=Q^T) so that the softmax key axis lands on the
    partition dim.  Q and K are passed pre-transposed [D, L] fp16 per batch.
  - ScalarE does ONLY the exp: P^T = exp(S^T/8), written fp16.  No mask bias:
    the sequence mask is folded into V' on the host instead (see below), so
    every exp instruction is identical and the Activation engine streams
    nothing but [128, 1024] exps back to back.
  - AV is orientation-swapped to make its matmuls cheap: for each 128-wide
    q chunk, O[q, 0:65] += matmul(lhsT=P^T[:, qc], rhs=V'[kb]) where
    V' = [V | 1] fp16 with ALL rows at key positions >= valid_len zeroed
    (including the ones column).  Masked keys therefore contribute exactly 0
    to both the weighted sum and the denominator, whatever exp produced for
    them.  Each AV matmul streams only 65 rows (vs 512 in the S^T
    orientation), so AV costs ~1/4 of QK on the PE.
  - Row 64 of each accumulator chunk is the softmax denominator; the epilogue
    is a per-partition-scalar normalize on VectorE (reciprocal of a [128, 4]
    column block, then 4 tensor_scalar multiplies) -- no partition broadcast
    needed because q is the partition dim of O.
  - Output is written fp16 in natural [L, D] layout (one DMA per slot) and
    upcast on the host.

Engine budget per (slot, key-block) unit: PE 1024 (QK) + 260 (AV) rows,
ScalarE 1024 exp columns + fixed overhead ~= 1040 ns.  The kernel is
Activation-bound; Q/K/V loads (fp16, ~390KB/slot) and output stores hide
under it.
"""

import math
from contextlib import ExitStack

import numpy as np

import concourse.bass as bass
import concourse.bacc as bacc
import concourse.mybir as mybir
import concourse.tile as tile
from concourse.bass_utils import run_bass_kernel_spmd

F32 = mybir.dt.float32
F16 = mybir.dt.float16
EXP = mybir.ActivationFunctionType.Exp

B, L, D = 64, 1024, 64
DV = D + 1            # V augmented with the ones/mask column
N_CORES = 8
SLOTS = B // N_CORES  # batches per core
KB = 128              # key-block size (partition dim of S^T)
N_KB = L // KB        # max key blocks
QH = 512              # q chunk per QK matmul (moving-operand max)
NQH = L // QH
NQC = L // KB         # q chunks of 128 for AV lhsT
SCALE = 1.0 / math.sqrt(D)


def build_kernel(counts):
    """counts[s] = number of 128-wide key blocks to process for slot s."""
    nc = bacc.Bacc()

    qt_d = nc.dram_tensor("qt", [SLOTS, D, L], F16, kind="ExternalInput")
    kt_d = nc.dram_tensor("kt", [SLOTS, D, L], F16, kind="ExternalInput")
    v_d = nc.dram_tensor("v", [SLOTS, KB, N_KB * DV], F16, kind="ExternalInput")
    out_d = nc.dram_tensor("out", [SLOTS, L, D], F16, kind="ExternalOutput")

    with tile.TileContext(nc) as tc, ExitStack() as ctx:
        const_pool = ctx.enter_context(tc.tile_pool(name="const", bufs=1))
        qt_pool = ctx.enter_context(tc.tile_pool(name="qt", bufs=2))
        kt_pool = ctx.enter_context(tc.tile_pool(name="kt", bufs=2))
        v_pool = ctx.enter_context(tc.tile_pool(name="v", bufs=3))
        p_pool = ctx.enter_context(tc.tile_pool(name="p", bufs=4))
        osb_pool = ctx.enter_context(tc.tile_pool(name="osb", bufs=2))
        rec_pool = ctx.enter_context(tc.tile_pool(name="rec", bufs=2))
        psum_s = ctx.enter_context(tc.tile_pool(name="psum_s", bufs=2, space="PSUM"))
        psum_o = ctx.enter_context(tc.tile_pool(name="psum_o", bufs=2, space="PSUM"))

        pair_tiles: dict[int, tuple] = {}
        v_tiles: dict[int, object] = {}
        pair_order = [1, 2, 3, 0]  # big pair last: tail epilogues hide in its loops
        slot_order = [2 * p + h for p in pair_order for h in range(2)]
        next_pair = {pair_order[i]: pair_order[i + 1] for i in range(len(pair_order) - 1)}

        def load_pair(p):
            if p in pair_tiles:
                return
            n_max = counts[2 * p]
            # Two batches packed on the partition dim: even batch in
            # partitions 0-63, odd batch in 64-127.
            qt_t = qt_pool.tile([2 * D, L], F16, name="qt_t")
            kt_t = kt_pool.tile([2 * D, L], F16, name="kt_t")
            src_q = qt_d[2 * p : 2 * p + 2].rearrange("b d l -> (b d) l")
            src_k = kt_d[2 * p : 2 * p + 2].rearrange("b d l -> (b d) l")
            if not pair_tiles:
                # Piecewise: the first slot's kb-0 QK only waits on the kt
                # head block + its own 64 qt rows.
                nc.sync.dma_start(kt_t[:, :KB], src_k[:, :KB])
                nc.sync.dma_start(qt_t[:D, :], src_q[:D, :])
                nc.sync.dma_start(qt_t[D:, :], src_q[D:, :])
                if n_max > 1:
                    nc.sync.dma_start(
                        kt_t[:, KB : n_max * KB], src_k[:, KB : n_max * KB]
                    )
            else:
                nc.sync.dma_start(qt_t[:], src_q)
                nc.sync.dma_start(kt_t[:, : n_max * KB], src_k[:, : n_max * KB])
            pair_tiles[p] = (qt_t, kt_t)

        def load_v(s):
            if s in v_tiles:
                return
            n_kb = counts[s]
            v_t = v_pool.tile([KB, N_KB * DV], F16, name="v_t")
            nc.gpsimd.dma_start(v_t[:, : n_kb * DV], v_d[s][:, : n_kb * DV])
            v_tiles[s] = v_t

        # Warm the exp table set while the first DMAs run (the explicit
        # table load otherwise lands right before the first real exp).
        warm_t = const_pool.tile([1, 1], F32)
        nc.scalar.activation(warm_t[:], warm_t[:], EXP)

        load_pair(pair_order[0])
        load_v(slot_order[0])
        load_v(slot_order[1])

        work = [(s, kb) for s in slot_order for kb in range(counts[s])]
        n_work = len(work)
        o_tiles: dict[int, object] = {}
        s_tiles: dict[tuple, object] = {}
        p_tiles: dict[tuple, object] = {}

        def emit_qk(i):
            s, kb = work[i]
            pair, half = divmod(s, 2)
            if kb == 0:
                # Slot prologue: prefetch upcoming inputs.
                nxt = slot_order.index(s) + 1
                if nxt + 1 < SLOTS:
                    load_v(slot_order[nxt + 1])
                if half == 0 and pair in next_pair:
                    load_pair(next_pair[pair])
            qt_t, kt_t = pair_tiles[pair]
            rows = slice(D * half, D * half + D)
            s_t = psum_s.tile([KB, L], F32, name="s_ps")
            s_tiles[(s, kb)] = s_t
            for qh in range(NQH):
                nc.tensor.matmul(
                    s_t[:, qh * QH : (qh + 1) * QH],
                    kt_t[rows, kb * KB : (kb + 1) * KB],
                    qt_t[rows, qh * QH : (qh + 1) * QH],
                    start=True,
                    stop=True,
                )

        def emit_av(i):
            # PSUM accumulation groups are 2KB-bank-granular (start=True marks
            # the whole bank pending-zero), so the O accumulator is laid out
            # [KB, 2 banks, 4 chunks, 128 (65 used)] with exactly one
            # start/stop per bank per slot; chunks 1-3 of each bank are
            # zeroed on first touch by the pending-zero mechanism.
            s, kb = work[i]
            n_kb = counts[s]
            if kb == 0:
                o_tiles[s] = psum_o.tile([KB, 2, NQC // 2, KB], F32, name="o_ps")
            o_ps = o_tiles[s]
            p_t = p_tiles.pop((s, kb))
            v_t = v_tiles[s]
            for qc in range(NQC):
                h, qcl = divmod(qc, NQC // 2)
                nc.tensor.matmul(
                    o_ps[:, h, qcl, :DV],
                    p_t[:, qc * KB : (qc + 1) * KB],
                    v_t[:, kb * DV : (kb + 1) * DV],
                    start=(kb == 0 and qcl == 0),
                    stop=(kb == n_kb - 1 and qcl == NQC // 2 - 1),
                    skip_group_check=True,
                )
            if kb == n_kb - 1:
                emit_epilogue(s)

        def emit_epilogue(s):
            o_ps = o_tiles.pop(s)
            rec = rec_pool.tile([KB, 2, NQC // 2, 1], F32, name="rec")
            out_sb = osb_pool.tile([KB, NQC, D], F16, name="out_sb")
            nc.vector.reciprocal(rec[:], o_ps[:, :, :, D : D + 1])
            for qc in range(NQC):
                h, qcl = divmod(qc, NQC // 2)
                nc.vector.tensor_scalar(
                    out_sb[:, qc, :],
                    o_ps[:, h, qcl, :D],
                    rec[:, h, qcl, :],
                    None,
                    mybir.AluOpType.mult,
                )
            nc.sync.dma_start(
                out_d[s].rearrange("(qc p) d -> p qc d", p=KB),
                out_sb[:],
            )

        emit_qk(0)
        for i in range(n_work):
            if i + 1 < n_work:
                emit_qk(i + 1)
            s, kb = work[i]
            p_t = p_pool.tile([KB, L], F16, name="p_t")
            p_tiles[(s, kb)] = p_t
            nc.scalar.activation(
                p_t[:], s_tiles.pop((s, kb))[:], EXP, scale=SCALE
            )
            if i >= 1:
                emit_av(i - 1)
        emit_av(n_work - 1)

    nc.finalize()
    return nc


_NC_CACHE: dict[tuple, object] = {}


def _prepare(queries, keys, values, valid_lens):
    queries = np.asarray(queries, dtype=np.float32)
    keys = np.asarray(keys, dtype=np.float32)
    values = np.asarray(values, dtype=np.float32)
    valid_lens = np.asarray(valid_lens)
    assert queries.shape == (B, L, D), queries.shape
    vl = valid_lens.astype(np.int64)

    # Sort batches by valid_len descending; slot s on core c gets the batch
    # of rank s*8 + c.  Each slot's loop count covers the max valid_len in
    # its rank group, so one instruction stream fits all cores.
    order = np.argsort(-vl, kind="stable")
    counts = tuple(
        max(1, math.ceil(int(vl[order[s * N_CORES]]) / KB)) for s in range(SLOTS)
    )
    nc = _NC_CACHE.get(counts)
    if nc is None:
        nc = build_kernel(counts)
        _NC_CACHE[counts] = nc

    col = np.arange(L)
    in_maps = []
    for c in range(N_CORES):
        batch_idx = [int(order[s * N_CORES + c]) for s in range(SLOTS)]
        qt = np.ascontiguousarray(
            queries[batch_idx].transpose(0, 2, 1).astype(np.float16)
        )  # [SLOTS, D, L]
        kt = np.ascontiguousarray(keys[batch_idx].transpose(0, 2, 1).astype(np.float16))
        # V' = [V | 1] with rows at masked key positions zeroed, laid out
        # [KB, N_KB * DV]: partition k%128, then (key block, d).
        v = np.concatenate(
            [values[batch_idx], np.ones((SLOTS, L, 1), np.float32)], axis=2
        )  # [SLOTS, L, DV]
        keep = (col[None, :] < vl[batch_idx][:, None]).astype(np.float32)
        v *= keep[:, :, None]
        v = np.ascontiguousarray(
            v.reshape(SLOTS, N_KB, KB, DV).transpose(0, 2, 1, 3).reshape(
                SLOTS, KB, N_KB * DV
            ).astype(np.float16)
        )
        in_maps.append({"qt": qt, "kt": kt, "v": v})
    return nc, in_maps, order


def _unshard(res, order):
    out = np.empty((B, L, D), dtype=np.float32)
    for c in range(N_CORES):
        o = res.results[c]["out"]  # [SLOTS, L, D] fp16
        for s in range(SLOTS):
            out[int(order[s * N_CORES + c])] = o[s].astype(np.float32)
    return out


def kernel(queries, keys, values, valid_lens):
    nc, in_maps, order = _prepare(queries, keys, values, valid_lens)
    res = run_bass_kernel_spmd(nc, in_maps, core_ids=list(range(N_CORES)))
    return _unshard(res, order)


def trace_run(queries, keys, values, valid_lens):
    """Like kernel() but traced; returns BassKernelResults (for test.py)."""
    nc, in_maps, order = _prepare(queries, keys, values, valid_lens)
    res = run_bass_kernel_spmd(
        nc, in_maps, core_ids=list(range(N_CORES)), trace=True
    )
    res.full_output = _unshard(res, order)
    return res


# revision 11
# speedup vs baseline: 1.1482x; 1.0394x over previous
"""Masked dot-product attention (B=64, L=1024, D=64, fp32) on 8 NeuronCores.

Strategy (data-parallel over batch, per the sharding hint):
  - Batches are sorted by valid_len (descending) and dealt round-robin to the
    8 cores so every core gets one batch from each of 8 "rank groups"; the
    per-slot key-block loop count is baked at build time as the max over that
    slot's rank group.  Key blocks that are entirely masked are never computed.
  - Scores are computed transposed, S^T[k, q] = K @ Q^T, via
    matmul(lhsT=K^T_slice, rhs=Q^T) so that the softmax key axis lands on the
    partition dim.  Q and K are passed pre-transposed [D, L] fp16 per batch.
  - ScalarE does ONLY the exp: P^T = exp(S^T/8), written fp16.  No mask bias:
    the sequence mask is folded into V' on the host instead (see below), so
    every exp instruction is identical and the Activation engine streams
    nothing but [128, 1024] exps back to back.
  - AV is orientation-swapped to make its matmuls cheap: for each 128-wide
    q chunk, O[q, 0:65] += matmul(lhsT=P^T[:, qc], rhs=V'[kb]) where
    V' = [V | 1] fp16 with ALL rows at key positions >= valid_len zeroed
    (including the ones column).  Masked keys therefore contribute exactly 0
    to both the weighted sum and the denominator, whatever exp produced for
    them.  Each AV matmul streams only 65 rows (vs 512 in the S^T
    orientation), so AV costs ~1/4 of QK on the PE.
  - Row 64 of each accumulator chunk is the softmax denominator; the epilogue
    is a per-partition-scalar normalize on VectorE (reciprocal of a [128, 4]
    column block, then 4 tensor_scalar multiplies) -- no partition broadcast
    needed because q is the partition dim of O.
  - Output is written fp16 in natural [L, D] layout (one DMA per slot) and
    upcast on the host.

Engine budget per (slot, key-block) unit: PE 1024 (QK) + 260 (AV) rows,
ScalarE 1024 exp columns + fixed overhead ~= 1040 ns.  The kernel is
Activation-bound; Q/K/V loads (fp16, ~390KB/slot) and output stores hide
under it.
"""

import math
from contextlib import ExitStack

import numpy as np

import concourse.bass as bass
import concourse.bacc as bacc
import concourse.mybir as mybir
import concourse.tile as tile
from concourse.bass_utils import run_bass_kernel_spmd

F32 = mybir.dt.float32
F16 = mybir.dt.float16
EXP = mybir.ActivationFunctionType.Exp

B, L, D = 64, 1024, 64
DV = D + 1            # V augmented with the ones/mask column
N_CORES = 8
SLOTS = B // N_CORES  # batches per core
KB = 128              # key-block size (partition dim of S^T)
N_KB = L // KB        # max key blocks
QH = 512              # q chunk per QK matmul (moving-operand max)
NQH = L // QH
NQC = L // KB         # q chunks of 128 for AV lhsT
SCALE = 1.0 / math.sqrt(D)


def build_kernel(counts):
    """counts[s] = number of 128-wide key blocks to process for slot s."""
    nc = bacc.Bacc()

    qt_d = nc.dram_tensor("qt", [SLOTS, D, L], F16, kind="ExternalInput")
    kt_d = nc.dram_tensor("kt", [SLOTS, D, L], F16, kind="ExternalInput")
    v_d = nc.dram_tensor("v", [SLOTS, KB, N_KB * DV], F16, kind="ExternalInput")
    # Output is partition-major [p, h, qcl, d] (q = (h*4+qcl)*128 + p) so the
    # store DMA has 1KB-contiguous runs on both sides; host restores [L, D].
    out_d = nc.dram_tensor(
        "out", [SLOTS, KB, 2, NQC // 2, D], F16, kind="ExternalOutput"
    )

    with tile.TileContext(nc) as tc, ExitStack() as ctx:
        const_pool = ctx.enter_context(tc.tile_pool(name="const", bufs=1))
        qt_pool = ctx.enter_context(tc.tile_pool(name="qt", bufs=2))
        kt_pool = ctx.enter_context(tc.tile_pool(name="kt", bufs=2))
        v_pool = ctx.enter_context(tc.tile_pool(name="v", bufs=3))
        p_pool = ctx.enter_context(tc.tile_pool(name="p", bufs=4))
        osb_pool = ctx.enter_context(tc.tile_pool(name="osb", bufs=2))
        rec_pool = ctx.enter_context(tc.tile_pool(name="rec", bufs=2))
        psum_s = ctx.enter_context(tc.tile_pool(name="psum_s", bufs=2, space="PSUM"))
        psum_o = ctx.enter_context(tc.tile_pool(name="psum_o", bufs=2, space="PSUM"))

        pair_tiles: dict[int, tuple] = {}
        v_tiles: dict[int, object] = {}
        pair_order = [1, 2, 3, 0]  # big pair last: tail epilogues hide in its loops
        slot_order = [2 * p + h for p in pair_order for h in range(2)]
        next_pair = {pair_order[i]: pair_order[i + 1] for i in range(len(pair_order) - 1)}

        def load_pair(p):
            if p in pair_tiles:
                return
            n_max = counts[2 * p]
            # Two batches packed on the partition dim: even batch in
            # partitions 0-63, odd batch in 64-127.
            qt_t = qt_pool.tile([2 * D, L], F16, name="qt_t")
            kt_t = kt_pool.tile([2 * D, L], F16, name="kt_t")
            src_q = qt_d[2 * p : 2 * p + 2].rearrange("b d l -> (b d) l")
            src_k = kt_d[2 * p : 2 * p + 2].rearrange("b d l -> (b d) l")
            if not pair_tiles:
                # Piecewise: the first slot's kb-0 QK only waits on the kt
                # head block + its own 64 qt rows.
                nc.sync.dma_start(kt_t[:, :KB], src_k[:, :KB])
                nc.sync.dma_start(qt_t[:D, :], src_q[:D, :])
                nc.sync.dma_start(qt_t[D:, :], src_q[D:, :])
                if n_max > 1:
                    nc.sync.dma_start(
                        kt_t[:, KB : n_max * KB], src_k[:, KB : n_max * KB]
                    )
            else:
                nc.sync.dma_start(qt_t[:], src_q)
                nc.sync.dma_start(kt_t[:, : n_max * KB], src_k[:, : n_max * KB])
            pair_tiles[p] = (qt_t, kt_t)

        def load_v(s):
            if s in v_tiles:
                return
            n_kb = counts[s]
            v_t = v_pool.tile([KB, N_KB * DV], F16, name="v_t")
            nc.gpsimd.dma_start(v_t[:, : n_kb * DV], v_d[s][:, : n_kb * DV])
            v_tiles[s] = v_t

        # Warm the exp table set while the first DMAs run (the explicit
        # table load otherwise lands right before the first real exp).
        warm_t = const_pool.tile([1, 1], F32)
        nc.scalar.activation(warm_t[:], warm_t[:], EXP)

        load_pair(pair_order[0])
        load_v(slot_order[0])
        load_v(slot_order[1])

        work = [(s, kb) for s in slot_order for kb in range(counts[s])]
        # Tail interleave: move the second-to-last slot's final key block to
        # the very end, so the last slot's epilogue (DVE + store DMA)
        # overlaps that unit instead of serializing after everything.
        s_a, s_b = slot_order[-2], slot_order[-1]
        if counts[s_a] > 1:
            last_a = work.index((s_a, counts[s_a] - 1))
            work.append(work.pop(last_a))
        n_work = len(work)
        o_tiles: dict[int, object] = {}
        s_tiles: dict[tuple, object] = {}
        p_tiles: dict[tuple, object] = {}

        def emit_qk(i):
            s, kb = work[i]
            pair, half = divmod(s, 2)
            if kb == 0:
                # Slot prologue: prefetch upcoming inputs.
                nxt = slot_order.index(s) + 1
                if nxt + 1 < SLOTS:
                    load_v(slot_order[nxt + 1])
                if half == 0 and pair in next_pair:
                    load_pair(next_pair[pair])
            qt_t, kt_t = pair_tiles[pair]
            rows = slice(D * half, D * half + D)
            s_t = psum_s.tile([KB, L], F32, name="s_ps")
            s_tiles[(s, kb)] = s_t
            for qh in range(NQH):
                nc.tensor.matmul(
                    s_t[:, qh * QH : (qh + 1) * QH],
                    kt_t[rows, kb * KB : (kb + 1) * KB],
                    qt_t[rows, qh * QH : (qh + 1) * QH],
                    start=True,
                    stop=True,
                )

        def emit_av(i):
            # PSUM accumulation groups are 2KB-bank-granular (start=True marks
            # the whole bank pending-zero), so the O accumulator is laid out
            # [KB, 2 banks, 4 chunks, 128 (65 used)] with exactly one
            # start/stop per bank per slot; chunks 1-3 of each bank are
            # zeroed on first touch by the pending-zero mechanism.
            s, kb = work[i]
            n_kb = counts[s]
            if kb == 0:
                o_tiles[s] = psum_o.tile([KB, 2, NQC // 2, KB], F32, name="o_ps")
            o_ps = o_tiles[s]
            p_t = p_tiles.pop((s, kb))
            v_t = v_tiles[s]
            for qc in range(NQC):
                h, qcl = divmod(qc, NQC // 2)
                nc.tensor.matmul(
                    o_ps[:, h, qcl, :DV],
                    p_t[:, qc * KB : (qc + 1) * KB],
                    v_t[:, kb * DV : (kb + 1) * DV],
                    start=(kb == 0 and qcl == 0),
                    stop=(kb == n_kb - 1 and qcl == NQC // 2 - 1),
                    skip_group_check=True,
                )
            if kb == n_kb - 1:
                emit_epilogue(s, split=(s == s_a))

        def emit_epilogue(s, split=False):
            o_ps = o_tiles.pop(s)
            rec = rec_pool.tile([KB, 2, NQC // 2], F32, name="rec")
            out_sb = osb_pool.tile([KB, 2, NQC // 2, D], F16, name="out_sb")
            nc.vector.reciprocal(rec[:], o_ps[:, :, :, D : D + 1])
            # One broadcast multiply: rec is read with a 0-stride last dim so
            # each chunk's denominator scales its whole 64-wide row.
            rec_ap = rec[:]
            rec_b = bass.AP(
                rec_ap.tensor, rec_ap.offset, list(rec_ap.ap) + [[0, D]]
            )
            if split:
                # Tail epilogue: per-half multiplies + stores so the first
                # half's DMA overlaps the second half's multiply.
                for h in range(2):
                    rh = rec[:, h]
                    nc.vector.tensor_tensor(
                        out_sb[:, h], o_ps[:, h, :, :D],
                        bass.AP(rh.tensor, rh.offset, list(rh.ap) + [[0, D]]),
                        op=mybir.AluOpType.mult,
                    )
                    nc.sync.dma_start(out_d[s][:, h], out_sb[:, h])
            else:
                nc.vector.tensor_tensor(
                    out_sb[:], o_ps[:, :, :, :D], rec_b,
                    op=mybir.AluOpType.mult,
                )
                nc.sync.dma_start(out_d[s][:], out_sb[:])

        emit_qk(0)
        for i in range(n_work):
            if i + 1 < n_work:
                emit_qk(i + 1)
            s, kb = work[i]
            p_t = p_pool.tile([KB, L], F16, name="p_t")
            p_tiles[(s, kb)] = p_t
            nc.scalar.activation(
                p_t[:], s_tiles.pop((s, kb))[:], EXP, scale=SCALE
            )
            if i >= 1:
                emit_av(i - 1)
        emit_av(n_work - 1)

    nc.finalize()
    return nc


_NC_CACHE: dict[tuple, object] = {}


def _prepare(queries, keys, values, valid_lens):
    queries = np.asarray(queries, dtype=np.float32)
    keys = np.asarray(keys, dtype=np.float32)
    values = np.asarray(values, dtype=np.float32)
    valid_lens = np.asarray(valid_lens)
    assert queries.shape == (B, L, D), queries.shape
    vl = valid_lens.astype(np.int64)

    # Sort batches by valid_len descending; slot s on core c gets the batch
    # of rank s*8 + c.  Each slot's loop count covers the max valid_len in
    # its rank group, so one instruction stream fits all cores.
    order = np.argsort(-vl, kind="stable")
    counts = tuple(
        max(1, math.ceil(int(vl[order[s * N_CORES]]) / KB)) for s in range(SLOTS)
    )
    nc = _NC_CACHE.get(counts)
    if nc is None:
        nc = build_kernel(counts)
        _NC_CACHE[counts] = nc

    col = np.arange(L)
    in_maps = []
    for c in range(N_CORES):
        batch_idx = [int(order[s * N_CORES + c]) for s in range(SLOTS)]
        qt = np.ascontiguousarray(
            queries[batch_idx].transpose(0, 2, 1).astype(np.float16)
        )  # [SLOTS, D, L]
        kt = np.ascontiguousarray(keys[batch_idx].transpose(0, 2, 1).astype(np.float16))
        # V' = [V | 1] with rows at masked key positions zeroed, laid out
        # [KB, N_KB * DV]: partition k%128, then (key block, d).
        v = np.concatenate(
            [values[batch_idx], np.ones((SLOTS, L, 1), np.float32)], axis=2
        )  # [SLOTS, L, DV]
        keep = (col[None, :] < vl[batch_idx][:, None]).astype(np.float32)
        v *= keep[:, :, None]
        v = np.ascontiguousarray(
            v.reshape(SLOTS, N_KB, KB, DV).transpose(0, 2, 1, 3).reshape(
                SLOTS, KB, N_KB * DV
            ).astype(np.float16)
        )
        in_maps.append({"qt": qt, "kt": kt, "v": v})
    return nc, in_maps, order


def _unshard(res, order):
    out = np.empty((B, L, D), dtype=np.float32)
    for c in range(N_CORES):
        o = res.results[c]["out"]  # [SLOTS, KB, 2, NQC//2, D] fp16
        for s in range(SLOTS):
            # q = (h * (NQC//2) + qcl) * KB + p
            out[int(order[s * N_CORES + c])] = (
                o[s].transpose(1, 2, 0, 3).reshape(L, D).astype(np.float32)
            )
    return out


def kernel(queries, keys, values, valid_lens):
    nc, in_maps, order = _prepare(queries, keys, values, valid_lens)
    res = run_bass_kernel_spmd(nc, in_maps, core_ids=list(range(N_CORES)))
    return _unshard(res, order)


def trace_run(queries, keys, values, valid_lens):
    """Like kernel() but traced; returns BassKernelResults (for test.py)."""
    nc, in_maps, order = _prepare(queries, keys, values, valid_lens)
    res = run_bass_kernel_spmd(
        nc, in_maps, core_ids=list(range(N_CORES)), trace=True
    )
    res.full_output = _unshard(res, order)
    return res


# revision 28
# speedup vs baseline: 1.2870x; 1.1209x over previous
"""Masked dot-product attention (B=64, L=1024, D=64, fp32) on 8 NeuronCores.

Same math as kernel.py (S^T = K Q^T per 128-key block, maskless exp with the
sequence mask folded into host-zeroed V' rows, orientation-swapped AV), but
the distributed work unit is a (batch, q-half) ITEM rather than a batch:

  - 128 items (each batch contributes two 512-wide q halves) are sorted by
    valid_len and dealt round-robin to the 8 cores; slot t runs at the max
    key-block count of its rank group.  Finer granularity tightens the SPMD
    balance: sum-of-slot-maxima drops ~7% vs whole-batch slots.
  - A work unit is (item, key block): QK is one [128, 512] matmul, AV is 4
    65-row matmuls into the item's [128, 4, 128] single-bank accumulator.
  - THREE consecutive units share one Activation instruction: their S tiles
    are one [128, 3, 512] PSUM tile (3 banks, one accumulation group per
    bank) and the exp processes [128, 1536] at once, amortizing the ~185ns
    per-activation fixed cost (27 exps instead of 43).
  - PSUM: 2 x 3-bank S tiles + 2 x 1-bank O tiles = 8 banks exactly.
"""

import math
from contextlib import ExitStack

import numpy as np

import concourse.bass as bass
import concourse.bacc as bacc
import concourse.mybir as mybir
import concourse.tile as tile
from concourse.bass_utils import run_bass_kernel_spmd

F32 = mybir.dt.float32
F16 = mybir.dt.float16
EXP = mybir.ActivationFunctionType.Exp

B, L, D = 64, 1024, 64
DV = D + 1
N_CORES = 8
KB = 128
N_KB = L // KB
QW = 512              # q width of one item (half a batch)
NQC = QW // KB        # 4 q chunks of 128 per item
N_ITEMS = 2 * B // N_CORES  # 16 item slots per core
N_PAIRS = N_ITEMS // 2
SCALE = 1.0 / math.sqrt(D)
MW = 3                # exp merge width (units per activation)

OPTS = {"head_widths": (1, 2), "tail_widths": (2, 1)}


def _merge_plan(n_units):
    """Merge widths: short ramps at head (DMA latency) and tail (epilogue
    chain after the final exp), MW-wide in the middle."""
    head = list(OPTS["head_widths"])
    tail = list(OPTS["tail_widths"])
    if sum(head) + sum(tail) > n_units:
        return [n_units]
    rem = n_units - sum(head) - sum(tail)
    mid = [MW] * (rem // MW)
    if rem % MW:
        mid.append(rem % MW)
    widths = head + mid + tail
    assert sum(widths) == n_units
    return widths


def build_kernel(counts):
    """counts[t] = key blocks for item slot t (t = 0..15, descending)."""
    nc = bacc.Bacc()

    qt_d = nc.dram_tensor("qt", [N_PAIRS, 2 * D, QW], F16, kind="ExternalInput")
    kt_d = nc.dram_tensor("kt", [N_PAIRS, 2 * D, L], F16, kind="ExternalInput")
    v_d = nc.dram_tensor("v", [N_ITEMS, KB, N_KB * DV], F16, kind="ExternalInput")
    out_d = nc.dram_tensor("out", [N_ITEMS, KB, NQC, D], F16, kind="ExternalOutput")

    pair_counts = [max(counts[2 * j], counts[2 * j + 1]) for j in range(N_PAIRS)]

    with tile.TileContext(nc) as tc, ExitStack() as ctx:
        const_pool = ctx.enter_context(tc.tile_pool(name="const", bufs=1))
        qt_pool = ctx.enter_context(tc.tile_pool(name="qt", bufs=2))
        kt_pool = ctx.enter_context(tc.tile_pool(name="kt", bufs=2))
        v_pool = ctx.enter_context(tc.tile_pool(name="v", bufs=4))
        p_pool = ctx.enter_context(tc.tile_pool(name="p", bufs=3))
        osb_pool = ctx.enter_context(tc.tile_pool(name="osb", bufs=6))
        rec_pool = ctx.enter_context(tc.tile_pool(name="rec", bufs=6))
        psum_s = ctx.enter_context(tc.tile_pool(name="psum_s", bufs=2, space="PSUM"))
        psum_o = ctx.enter_context(tc.tile_pool(name="psum_o", bufs=2, space="PSUM"))

        pair_tiles: dict[int, tuple] = {}
        v_tiles: dict[int, object] = {}

        def load_pair(j):
            if j in pair_tiles:
                return
            n_max = pair_counts[j]
            qt_t = qt_pool.tile([2 * D, QW], F16, name="qt_t")
            kt_t = kt_pool.tile([2 * D, L], F16, name="kt_t")
            if not pair_tiles:
                # First pair: item 0's qt rides the Activation HWDGE queue
                # so its issue overlaps the SP queue's; item 0's whole K row block
                # comes as one DMA (one +900ns completion chain instead of
                # three); item 1's rows follow on SP.
                nc.scalar.dma_start(qt_t[:D, :], qt_d[j][:D, :])
                nc.sync.dma_start(kt_t[:D, :KB], kt_d[j][:D, :KB])
                if counts[0] > 1:
                    nc.sync.dma_start(
                        kt_t[:D, KB : counts[0] * KB],
                        kt_d[j][:D, KB : counts[0] * KB],
                    )
                nc.sync.dma_start(qt_t[D:, :], qt_d[j][D:, :])
                nc.sync.dma_start(kt_t[D:, :KB], kt_d[j][D:, :KB])
                if n_max > 1:
                    nc.sync.dma_start(
                        kt_t[D:, KB : n_max * KB], kt_d[j][D:, KB : n_max * KB]
                    )
            else:
                nc.sync.dma_start(qt_t[:], qt_d[j][:])
                nc.sync.dma_start(kt_t[:, : n_max * KB], kt_d[j][:, : n_max * KB])
            pair_tiles[j] = (qt_t, kt_t)

        def load_v(t, engine=None):
            if t in v_tiles:
                return
            n_kb = counts[t]
            v_t = v_pool.tile([KB, N_KB * DV], F16, name="v_t")
            (engine or nc.gpsimd).dma_start(v_t[:, : n_kb * DV], v_d[t][:, : n_kb * DV])
            v_tiles[t] = v_t

        # Warm the exp table while the first loads run.
        warm_t = const_pool.tile([1, 1], F32)
        nc.scalar.activation(warm_t[:], warm_t[:], EXP)

        load_pair(0)
        load_v(0, engine=nc.sync)
        load_v(1, engine=nc.sync)

        units = [(t, kb) for t in range(N_ITEMS) for kb in range(counts[t])]
        widths = _merge_plan(len(units))
        merges = []
        u = 0
        for w in widths:
            merges.append(units[u : u + w])
            u += w
        n_m = len(merges)

        o_tiles: dict[int, object] = {}
        s_tiles: dict[int, object] = {}
        p_tiles: dict[int, object] = {}

        def emit_qk(i):
            group = merges[i]
            s_t = psum_s.tile([KB, MW, QW], F32, name="s_ps")
            s_tiles[i] = s_t
            for m, (t, kb) in enumerate(group):
                if kb == 0:
                    # Item prologue: prefetch upcoming inputs.
                    if t + 2 < N_ITEMS:
                        load_v(t + 2)
                    pair, half = divmod(t, 2)
                    if half == 0 and pair + 1 < N_PAIRS:
                        load_pair(pair + 1)
                pair, half = divmod(t, 2)
                qt_t, kt_t = pair_tiles[pair]
                rows = slice(D * half, D * half + D)
                nc.tensor.matmul(
                    s_t[:, m, :],
                    kt_t[rows, kb * KB : (kb + 1) * KB],
                    qt_t[rows, :],
                    start=True,
                    stop=True,
                )

        def emit_exp(i):
            group = merges[i]
            w = len(group)
            p_t = p_pool.tile([KB, MW, QW], F16, name="p_t")
            p_tiles[i] = p_t
            nc.scalar.activation(
                p_t[:, :w, :], s_tiles.pop(i)[:, :w, :], EXP, scale=SCALE
            )

        def emit_av(i):
            group = merges[i]
            p_t = p_tiles.pop(i)
            for m, (t, kb) in enumerate(group):
                n_kb = counts[t]
                if kb == 0:
                    o_tiles[t] = psum_o.tile([KB, NQC, KB], F32, name="o_ps")
                o_ps = o_tiles[t]
                v_t = v_tiles[t]
                for qc in range(NQC):
                    nc.tensor.matmul(
                        o_ps[:, qc, :DV],
                        p_t[:, m, qc * KB : (qc + 1) * KB],
                        v_t[:, kb * DV : (kb + 1) * DV],
                        start=(kb == 0 and qc == 0),
                        stop=(kb == n_kb - 1 and qc == NQC - 1),
                        skip_group_check=True,
                    )
                if kb == n_kb - 1:
                    emit_epilogue(t)

        def emit_epilogue(t):
            o_ps = o_tiles.pop(t)
            rec = rec_pool.tile([KB, NQC], F32, name="rec")
            out_sb = osb_pool.tile([KB, NQC, D], F16, name="out_sb")
            rec_ap = rec[:]
            nc.vector.reciprocal(rec[:], o_ps[:, :, D : D + 1])
            nc.vector.tensor_tensor(
                out_sb[:], o_ps[:, :, :D],
                bass.AP(rec_ap.tensor, rec_ap.offset, list(rec_ap.ap) + [[0, D]]),
                op=mybir.AluOpType.mult,
            )
            nc.sync.dma_start(out_d[t][:], out_sb[:])

        emit_qk(0)
        for i in range(n_m):
            if i + 1 < n_m:
                emit_qk(i + 1)
            emit_exp(i)
            if i >= 1:
                emit_av(i - 1)
        emit_av(n_m - 1)

    nc.finalize()
    return nc


_NC_CACHE: dict[tuple, object] = {}


def _prepare(queries, keys, values, valid_lens):
    queries = np.asarray(queries, dtype=np.float32)
    keys = np.asarray(keys, dtype=np.float32)
    values = np.asarray(values, dtype=np.float32)
    valid_lens = np.asarray(valid_lens)
    assert queries.shape == (B, L, D), queries.shape
    vl = valid_lens.astype(np.int64)

    # Items: (batch, q-half), sorted by valid_len desc; item rank r goes to
    # core r % 8, slot r // 8.  Slot t's key-block count covers the max
    # valid_len in its rank group (= rank 8t), so one SPMD stream fits all.
    vl_items = np.repeat(vl, 2)
    order = np.argsort(-vl_items, kind="stable")  # item id = 2*batch + half
    counts = tuple(
        max(1, math.ceil(int(vl_items[order[t * N_CORES]]) / KB))
        for t in range(N_ITEMS)
    )
    nc = _NC_CACHE.get(counts)
    if nc is None:
        nc = build_kernel(counts)
        _NC_CACHE[counts] = nc

    col = np.arange(L)
    in_maps = []
    for c in range(N_CORES):
        item_ids = [int(order[t * N_CORES + c]) for t in range(N_ITEMS)]
        batches = [i // 2 for i in item_ids]
        halves = [i % 2 for i in item_ids]
        # qt: [pair, 2*D, QW] -- item t occupies rows (t%2)*64 +- 64.
        qt = np.empty((N_PAIRS, 2 * D, QW), np.float16)
        kt = np.empty((N_PAIRS, 2 * D, L), np.float16)
        for t in range(N_ITEMS):
            b, h = batches[t], halves[t]
            j, half = divmod(t, 2)
            qt[j, half * D : half * D + D] = (
                queries[b, h * QW : (h + 1) * QW].T.astype(np.float16)
            )
            kt[j, half * D : half * D + D] = keys[b].T.astype(np.float16)
        v = np.concatenate(
            [values[batches], np.ones((N_ITEMS, L, 1), np.float32)], axis=2
        )
        keep = (col[None, :] < vl[batches][:, None]).astype(np.float32)
        v *= keep[:, :, None]
        v = np.ascontiguousarray(
            v.reshape(N_ITEMS, N_KB, KB, DV).transpose(0, 2, 1, 3).reshape(
                N_ITEMS, KB, N_KB * DV
            ).astype(np.float16)
        )
        in_maps.append({"qt": qt, "kt": kt, "v": v})
    return nc, in_maps, order


def _unshard(res, order):
    out = np.empty((B, L, D), dtype=np.float32)
    for c in range(N_CORES):
        o = res.results[c]["out"]  # [N_ITEMS, KB, NQC, D] fp16
        for t in range(N_ITEMS):
            item = int(order[t * N_CORES + c])
            b, h = divmod(item, 2)
            # q = h*QW + qc*KB + p
            out[b, h * QW : (h + 1) * QW] = (
                o[t].transpose(1, 0, 2).reshape(QW, D).astype(np.float32)
            )
    return out


def kernel(queries, keys, values, valid_lens):
    nc, in_maps, order = _prepare(queries, keys, values, valid_lens)
    res = run_bass_kernel_spmd(nc, in_maps, core_ids=list(range(N_CORES)))
    return _unshard(res, order)


def trace_run(queries, keys, values, valid_lens):
    nc, in_maps, order = _prepare(queries, keys, values, valid_lens)
    res = run_bass_kernel_spmd(
        nc, in_maps, core_ids=list(range(N_CORES)), trace=True
    )
    res.full_output = _unshard(res, order)
    return res


# revision 29
# speedup vs baseline: 1.2928x; 1.0045x over previous
"""Masked dot-product attention (B=64, L=1024, D=64, fp32) on 8 NeuronCores.

Same math as kernel.py (S^T = K Q^T per 128-key block, maskless exp with the
sequence mask folded into host-zeroed V' rows, orientation-swapped AV), but
the distributed work unit is a (batch, q-half) ITEM rather than a batch:

  - 128 items (each batch contributes two 512-wide q halves) are sorted by
    valid_len and dealt round-robin to the 8 cores; slot t runs at the max
    key-block count of its rank group.  Finer granularity tightens the SPMD
    balance: sum-of-slot-maxima drops ~7% vs whole-batch slots.
  - A work unit is (item, key block): QK is one [128, 512] matmul, AV is 4
    65-row matmuls into the item's [128, 4, 128] single-bank accumulator.
  - THREE consecutive units share one Activation instruction: their S tiles
    are one [128, 3, 512] PSUM tile (3 banks, one accumulation group per
    bank) and the exp processes [128, 1536] at once, amortizing the ~185ns
    per-activation fixed cost (27 exps instead of 43).
  - PSUM: 2 x 3-bank S tiles + 2 x 1-bank O tiles = 8 banks exactly.
"""

import math
from contextlib import ExitStack

import numpy as np

import concourse.bass as bass
import concourse.bacc as bacc
import concourse.mybir as mybir
import concourse.tile as tile
from concourse.bass_utils import run_bass_kernel_spmd

F32 = mybir.dt.float32
F16 = mybir.dt.float16
EXP = mybir.ActivationFunctionType.Exp

B, L, D = 64, 1024, 64
DV = D + 1
N_CORES = 8
KB = 128
N_KB = L // KB
QW = 512              # q width of one item (half a batch)
NQC = QW // KB        # 4 q chunks of 128 per item
N_ITEMS = 2 * B // N_CORES  # 16 item slots per core
N_PAIRS = N_ITEMS // 2
SCALE = 1.0 / math.sqrt(D)
MW = 3                # exp merge width (units per activation)

OPTS = {"head_widths": (1, 2), "tail_widths": (2, 1)}


def _merge_plan(n_units):
    """Merge widths: short ramps at head (DMA latency) and tail (epilogue
    chain after the final exp), MW-wide in the middle."""
    head = list(OPTS["head_widths"])
    tail = list(OPTS["tail_widths"])
    if sum(head) + sum(tail) > n_units:
        return [n_units]
    rem = n_units - sum(head) - sum(tail)
    mid = [MW] * (rem // MW)
    if rem % MW:
        mid.append(rem % MW)
    widths = head + mid + tail
    assert sum(widths) == n_units
    return widths


def build_kernel(counts):
    """counts[t] = key blocks for item slot t (t = 0..15, descending)."""
    nc = bacc.Bacc()

    qt_d = nc.dram_tensor("qt", [N_PAIRS, 2 * D, QW], F16, kind="ExternalInput")
    kt_d = nc.dram_tensor("kt", [N_PAIRS, 2 * D, L], F16, kind="ExternalInput")
    v_d = nc.dram_tensor("v", [N_ITEMS, KB, N_KB * DV], F16, kind="ExternalInput")
    out_d = nc.dram_tensor("out", [N_ITEMS, KB, NQC, D], F16, kind="ExternalOutput")

    pair_counts = [max(counts[2 * j], counts[2 * j + 1]) for j in range(N_PAIRS)]

    with tile.TileContext(nc) as tc, ExitStack() as ctx:
        const_pool = ctx.enter_context(tc.tile_pool(name="const", bufs=1))
        qt_pool = ctx.enter_context(tc.tile_pool(name="qt", bufs=2))
        kt_pool = ctx.enter_context(tc.tile_pool(name="kt", bufs=2))
        v_pool = ctx.enter_context(tc.tile_pool(name="v", bufs=4))
        p_pool = ctx.enter_context(tc.tile_pool(name="p", bufs=3))
        osb_pool = ctx.enter_context(tc.tile_pool(name="osb", bufs=6))
        rec_pool = ctx.enter_context(tc.tile_pool(name="rec", bufs=6))
        psum_s = ctx.enter_context(tc.tile_pool(name="psum_s", bufs=2, space="PSUM"))
        psum_o = ctx.enter_context(tc.tile_pool(name="psum_o", bufs=2, space="PSUM"))

        pair_tiles: dict[int, tuple] = {}
        v_tiles: dict[int, object] = {}

        def load_pair(j):
            if j in pair_tiles:
                return
            n_max = pair_counts[j]
            qt_t = qt_pool.tile([2 * D, QW], F16, name="qt_t")
            kt_t = kt_pool.tile([2 * D, L], F16, name="kt_t")
            if not pair_tiles:
                # First pair: item 0's qt rides the Activation HWDGE queue
                # so its issue overlaps the SP queue's; item 0's whole K row
                # block comes as one DMA; item 1's rows follow on SP.
                nc.scalar.dma_start(qt_t[:D, :], qt_d[j][:D, :])
                nc.sync.dma_start(kt_t[:D, :KB], kt_d[j][:D, :KB])
                if counts[0] > 1:
                    nc.sync.dma_start(
                        kt_t[:D, KB : counts[0] * KB],
                        kt_d[j][:D, KB : counts[0] * KB],
                    )
                nc.sync.dma_start(qt_t[D:, :], qt_d[j][D:, :])
                nc.sync.dma_start(kt_t[D:, :KB], kt_d[j][D:, :KB])
                if n_max > 1:
                    nc.sync.dma_start(
                        kt_t[D:, KB : n_max * KB], kt_d[j][D:, KB : n_max * KB]
                    )
            else:
                nc.sync.dma_start(qt_t[:], qt_d[j][:])
                nc.sync.dma_start(kt_t[:, : n_max * KB], kt_d[j][:, : n_max * KB])
            pair_tiles[j] = (qt_t, kt_t)

        def load_v(t, engine=None, split=False):
            if t in v_tiles:
                return
            n_kb = counts[t]
            v_t = v_pool.tile([KB, N_KB * DV], F16, name="v_t")
            eng = engine or nc.gpsimd
            if split and n_kb > 1:
                # Halved so a single long transfer can't delay the critical
                # first qt/kt pieces on the serialized DMA engines.
                h = (n_kb // 2) * DV
                eng.dma_start(v_t[:, :h], v_d[t][:, :h])
                eng.dma_start(v_t[:, h : n_kb * DV], v_d[t][:, h : n_kb * DV])
            else:
                eng.dma_start(v_t[:, : n_kb * DV], v_d[t][:, : n_kb * DV])
            v_tiles[t] = v_t

        # Warm the exp table while the first loads run.
        warm_t = const_pool.tile([1, 1], F32)
        nc.scalar.activation(warm_t[:], warm_t[:], EXP)

        load_pair(0)
        load_v(0, engine=nc.sync, split=True)
        load_v(1, engine=nc.sync, split=True)

        units = [(t, kb) for t in range(N_ITEMS) for kb in range(counts[t])]
        widths = _merge_plan(len(units))
        merges = []
        u = 0
        for w in widths:
            merges.append(units[u : u + w])
            u += w
        n_m = len(merges)

        o_tiles: dict[int, object] = {}
        s_tiles: dict[int, object] = {}
        p_tiles: dict[int, object] = {}

        def emit_qk(i):
            group = merges[i]
            s_t = psum_s.tile([KB, MW, QW], F32, name="s_ps")
            s_tiles[i] = s_t
            for m, (t, kb) in enumerate(group):
                if kb == 0:
                    # Item prologue: prefetch upcoming inputs.
                    if t + 2 < N_ITEMS:
                        load_v(t + 2)
                    pair, half = divmod(t, 2)
                    if half == 0 and pair + 1 < N_PAIRS:
                        load_pair(pair + 1)
                pair, half = divmod(t, 2)
                qt_t, kt_t = pair_tiles[pair]
                rows = slice(D * half, D * half + D)
                nc.tensor.matmul(
                    s_t[:, m, :],
                    kt_t[rows, kb * KB : (kb + 1) * KB],
                    qt_t[rows, :],
                    start=True,
                    stop=True,
                )

        def emit_exp(i):
            group = merges[i]
            w = len(group)
            p_t = p_pool.tile([KB, MW, QW], F16, name="p_t")
            p_tiles[i] = p_t
            nc.scalar.activation(
                p_t[:, :w, :], s_tiles.pop(i)[:, :w, :], EXP, scale=SCALE
            )

        def emit_av(i):
            group = merges[i]
            p_t = p_tiles.pop(i)
            for m, (t, kb) in enumerate(group):
                n_kb = counts[t]
                if kb == 0:
                    o_tiles[t] = psum_o.tile([KB, NQC, KB], F32, name="o_ps")
                o_ps = o_tiles[t]
                v_t = v_tiles[t]
                for qc in range(NQC):
                    nc.tensor.matmul(
                        o_ps[:, qc, :DV],
                        p_t[:, m, qc * KB : (qc + 1) * KB],
                        v_t[:, kb * DV : (kb + 1) * DV],
                        start=(kb == 0 and qc == 0),
                        stop=(kb == n_kb - 1 and qc == NQC - 1),
                        skip_group_check=True,
                    )
                if kb == n_kb - 1:
                    emit_epilogue(t)

        def emit_epilogue(t):
            o_ps = o_tiles.pop(t)
            rec = rec_pool.tile([KB, NQC], F32, name="rec")
            out_sb = osb_pool.tile([KB, NQC, D], F16, name="out_sb")
            rec_ap = rec[:]
            nc.vector.reciprocal(rec[:], o_ps[:, :, D : D + 1])
            nc.vector.tensor_tensor(
                out_sb[:], o_ps[:, :, :D],
                bass.AP(rec_ap.tensor, rec_ap.offset, list(rec_ap.ap) + [[0, D]]),
                op=mybir.AluOpType.mult,
            )
            nc.sync.dma_start(out_d[t][:], out_sb[:])

        emit_qk(0)
        for i in range(n_m):
            if i + 1 < n_m:
                emit_qk(i + 1)
            emit_exp(i)
            if i >= 1:
                emit_av(i - 1)
        emit_av(n_m - 1)

    nc.finalize()
    return nc


_NC_CACHE: dict[tuple, object] = {}


def _prepare(queries, keys, values, valid_lens):
    queries = np.asarray(queries, dtype=np.float32)
    keys = np.asarray(keys, dtype=np.float32)
    values = np.asarray(values, dtype=np.float32)
    valid_lens = np.asarray(valid_lens)
    assert queries.shape == (B, L, D), queries.shape
    vl = valid_lens.astype(np.int64)

    # Items: (batch, q-half), sorted by valid_len desc; item rank r goes to
    # core r % 8, slot r // 8.  Slot t's key-block count covers the max
    # valid_len in its rank group (= rank 8t), so one SPMD stream fits all.
    vl_items = np.repeat(vl, 2)
    order = np.argsort(-vl_items, kind="stable")  # item id = 2*batch + half
    counts = tuple(
        max(1, math.ceil(int(vl_items[order[t * N_CORES]]) / KB))
        for t in range(N_ITEMS)
    )
    nc = _NC_CACHE.get(counts)
    if nc is None:
        nc = build_kernel(counts)
        _NC_CACHE[counts] = nc

    col = np.arange(L)
    in_maps = []
    for c in range(N_CORES):
        item_ids = [int(order[t * N_CORES + c]) for t in range(N_ITEMS)]
        batches = [i // 2 for i in item_ids]
        halves = [i % 2 for i in item_ids]
        # qt: [pair, 2*D, QW] -- item t occupies rows (t%2)*64 +- 64.
        qt = np.empty((N_PAIRS, 2 * D, QW), np.float16)
        kt = np.empty((N_PAIRS, 2 * D, L), np.float16)
        for t in range(N_ITEMS):
            b, h = batches[t], halves[t]
            j, half = divmod(t, 2)
            qt[j, half * D : half * D + D] = (
                queries[b, h * QW : (h + 1) * QW].T.astype(np.float16)
            )
            kt[j, half * D : half * D + D] = keys[b].T.astype(np.float16)
        v = np.concatenate(
            [values[batches], np.ones((N_ITEMS, L, 1), np.float32)], axis=2
        )
        keep = (col[None, :] < vl[batches][:, None]).astype(np.float32)
        v *= keep[:, :, None]
        v = np.ascontiguousarray(
            v.reshape(N_ITEMS, N_KB, KB, DV).transpose(0, 2, 1, 3).reshape(
                N_ITEMS, KB, N_KB * DV
            ).astype(np.float16)
        )
        in_maps.append({"qt": qt, "kt": kt, "v": v})
    return nc, in_maps, order


def _unshard(res, order):
    out = np.empty((B, L, D), dtype=np.float32)
    for c in range(N_CORES):
        o = res.results[c]["out"]  # [N_ITEMS, KB, NQC, D] fp16
        for t in range(N_ITEMS):
            item = int(order[t * N_CORES + c])
            b, h = divmod(item, 2)
            # q = h*QW + qc*KB + p
            out[b, h * QW : (h + 1) * QW] = (
                o[t].transpose(1, 0, 2).reshape(QW, D).astype(np.float32)
            )
    return out


def kernel(queries, keys, values, valid_lens):
    nc, in_maps, order = _prepare(queries, keys, values, valid_lens)
    res = run_bass_kernel_spmd(nc, in_maps, core_ids=list(range(N_CORES)))
    return _unshard(res, order)


def trace_run(queries, keys, values, valid_lens):
    nc, in_maps, order = _prepare(queries, keys, values, valid_lens)
    res = run_bass_kernel_spmd(
        nc, in_maps, core_ids=list(range(N_CORES)), trace=True
    )
    res.full_output = _unshard(res, order)
    return res
